# revision 23
# baseline (speedup 1.0000x reference)
"""3-layer GCN (gcn_norm message passing) on 8 Trainium2 NeuronCores.

Architecture (v8):
  - Nodes row-sharded across 8 cores (12500 real + 44 pad rows each); per
    layer each core computes h_mm = relu(h_prev) @ W for its shard, scaled by
    dis[src] (norm factorization: norm = dis[dest]*dis[src]), AllGathers the
    bf16 table, then aggregates messages for the destinations it owns.
  - The per-layer AllGather is split into 4 source windows of [28,28,28,14]
    blocks (int16 gather-index range bounds a window at 8*31 blocks); window
    w's collective fires mid-previous-layer once its table rows are written,
    with enough group slack that the trigger never stalls the gpsimd queue.
    Gathers for window w wait only on window w's collective; the tail
    window is small to soften layer boundaries.
  - The SWDGE gather desc-gen pipe sustains ~2.8ns/descriptor regardless of
    call size; calls are per (PSUM group, window) (matches compute
    granularity), with a window-major pre-issue of the first 3 groups at
    each layer start for pipe runway while tail-window collectives land.
  - Self-loops are NOT gathered: each block's own table rows are added into
    the aggregation PSUM with an identity matmul opening that block's PSUM
    bracket (hm rows are already scaled by dis[src]; the epilogue's
    *dis[dest] completes the dis^2 self norm).
  - Segment-sum on the TensorEngine: each 128-message chunk contributes one
    N=128 matmul per (statically known) destination block it overlaps, into
    a [128, 512] group PSUM tile with strictly sequential per-block
    accumulation brackets; one-hots are 0/1 bf16 built 8 chunks at a time
    with a single wide DVE tensor_tensor against a bf16 iota.
  - Split epilogue pipelining: group g's scale/bias/h_out run one group
    later and relu/next-layer-matmul/hm two groups later, so the in-order
    DVE/PE/ACT queues never convoy (one-hot builds for g+1 are queued ahead
    of g's PSUM-dependent ops). Tail groups use an immediate epilogue so the
    next layer's collectives see their table rows promptly.
  - gidx and meta live in SBUF (loaded once, sliced per call).

All data-dependent structure is baked at trace time; the NEFF is compiled
per call and cached in-process.
"""

import os
import sys

sys.path.insert(0, "/opt/trn_rl_repo")

import numpy as np
import ml_dtypes

from concourse import bacc, bass, mybir
from concourse import tile
from concourse import bass_utils

F32 = mybir.dt.float32
BF16 = mybir.dt.bfloat16
I16 = mybir.dt.int16

N_CORES = 8
G = 4        # dest blocks per PSUM group
WOH = 8      # one-hot chunks per wide DVE op
RUN = 32     # run alignment granularity (slots)
PAD_SEG = 10000.0
WIN_BLKS = [28, 28, 28, 14]   # source blocks per window (sum = nblk)
NW = len(WIN_BLKS)
K_PRE = 4                      # groups pre-issued at each layer start

SCRATCH = int(os.environ.get("TRN_SCRATCH", "32768"))
GATH_BUFS = int(os.environ.get("TRN_GATH_BUFS", "16"))


def _schedule(caps32, ngrp, nblk):
    """Static layout shared by prep and builder.

    caps32: [nblk][NW] per-(block, window) run capacity in RUN-slot units.
    Returns per-call offsets and the chunk->block matmul schedule.
    """
    call_cols = np.zeros(ngrp * NW + 1, dtype=np.int64)   # gidx col base
    chunk_base = np.zeros(ngrp * NW + 1, dtype=np.int64)  # chunk id base
    call_nidx = []
    run_slot = {}   # (b, w) -> slot offset of run inside its call
    mm_of_group = []
    for g in range(ngrp):
        blocks = list(range(g * G, min((g + 1) * G, nblk)))
        mms = []
        for w in range(NW):
            off = 0
            spans = []
            for b in blocks:
                run_slot[(b, w)] = off
                spans.append((b, off, off + caps32[b][w] * RUN))
                off += caps32[b][w] * RUN
            nidx = ((off + 127) // 128) * 128
            call_nidx.append(nidx)
            call_cols[g * NW + w + 1] = call_cols[g * NW + w] + nidx // 16
            chunk_base[g * NW + w + 1] = chunk_base[g * NW + w] + nidx // 128
            for c in range(nidx // 128):
                lo, hi = c * 128, (c + 1) * 128
                for b, s0, s1 in spans:
                    if s0 < hi and s1 > lo:
                        mms.append((b - g * G, w, c))
        # j-major order for PSUM accumulation bracketing
        mms.sort(key=lambda m: (m[0], m[1], m[2]))
        mm_of_group.append(mms)
    mm_base = np.zeros(ngrp + 1, dtype=np.int64)
    np.cumsum([len(m) for m in mm_of_group], out=mm_base[1:])
    return {
        "call_cols": call_cols, "chunk_base": chunk_base,
        "call_nidx": call_nidx, "run_slot": run_slot,
        "mm_of_group": mm_of_group, "mm_base": mm_base,
        "n_mm": int(mm_base[-1]),
        "gidx_cols": int(call_cols[-1]),
        "total_chunks": int(chunk_base[-1]),
    }


# ----------------------------------------------------------------------------
# Host-side preparation
# ----------------------------------------------------------------------------

def _prep_inputs(x, edge_index, W0, b0, W1, b1, W2, b2, s_real):
    n = x.shape[0]
    assert n % N_CORES == 0 and s_real == n // N_CORES
    nblk = (s_real + 127) // 128
    s_pad = nblk * 128
    ngrp = (nblk + G - 1) // G
    assert sum(WIN_BLKS) == nblk
    win_rows = [c * 128 for c in WIN_BLKS]
    cbr = np.concatenate([[0], np.cumsum(win_rows)])
    cbb = np.concatenate([[0], np.cumsum(WIN_BLKS)])
    assert all(N_CORES * r <= 32767 for r in win_rows)
    assert all(b % G == 0 for b in cbb[:-1])

    d = np.asarray(edge_index[0], dtype=np.int64)
    s = np.asarray(edge_index[1], dtype=np.int64)

    deg = np.bincount(s, minlength=n).astype(np.float64) + 1.0
    dis = (1.0 / np.sqrt(deg)).astype(np.float32)

    core = d // s_real
    dloc = d - core * s_real
    blk = dloc >> 7
    grp = blk // G
    jj = blk - grp * G
    score = s // s_real
    sloc = s - score * s_real
    sblk = sloc >> 7
    blk2win = np.zeros(nblk, dtype=np.int64)
    for w in range(NW):
        blk2win[cbb[w]:cbb[w + 1]] = w
    q = blk2win[sblk]
    widx = (
        score * np.asarray(win_rows)[q] + (sloc - cbr[q])
    ).astype(np.int64)

    # per-(core, block, window) counts -> shared run capacities (RUN units)
    key = (core * nblk + blk) * NW + q
    counts = np.bincount(key, minlength=N_CORES * nblk * NW).reshape(
        N_CORES, nblk, NW
    )
    caps32 = np.maximum(
        (counts.max(axis=0) + RUN - 1) // RUN, 1
    )  # [nblk, NW]

    lay = _schedule(caps32.tolist(), ngrp, nblk)

    # rank within (core, b, w)
    order = np.argsort(key, kind="stable")
    inv = np.empty_like(order)
    inv[order] = np.arange(order.size)
    starts = np.zeros(N_CORES * nblk * NW + 1, dtype=np.int64)
    np.cumsum(counts.reshape(-1), out=starts[1:])
    rank = inv - starts[key]

    run_slot_arr = np.zeros((nblk, NW), dtype=np.int64)
    for (b, ww), v in lay["run_slot"].items():
        run_slot_arr[b, ww] = v

    slot = run_slot_arr[blk, q] + rank               # slot within call
    call_id = grp * NW + q
    gcol = lay["call_cols"][call_id] + (slot >> 4)
    grow = slot & 15
    gchunk = lay["chunk_base"][call_id] + (slot >> 7)
    part = slot & 127

    gidx16 = np.zeros((N_CORES, 16, lay["gidx_cols"]), dtype=np.int16)
    gidx16[core, grow, gcol] = widx.astype(np.int16)
    gidx = np.broadcast_to(
        gidx16[:, None, :, :], (N_CORES, 8, 16, lay["gidx_cols"])
    ).reshape(N_CORES, 128, lay["gidx_cols"]).copy()

    # mm col lookup: (gchunk, j) -> column
    mm_col = np.full((lay["total_chunks"], G), -1, dtype=np.int64)
    cb = lay["chunk_base"]
    for g in range(ngrp):
        m0 = lay["mm_base"][g]
        for k, (j, ww, c) in enumerate(lay["mm_of_group"][g]):
            mm_col[cb[g * NW + ww] + c, j] = m0 + k

    meta = np.full((N_CORES, 128, lay["n_mm"]), PAD_SEG, dtype=np.float32)
    col = mm_col[gchunk, jj]
    assert (col >= 0).all()
    meta[core, part, col] = (dloc - blk * 128).astype(np.float32)
    meta = meta.astype(ml_dtypes.bfloat16)

    # dense inputs
    x = np.asarray(x, dtype=np.float32)
    x_t = np.zeros((N_CORES, 128, s_pad), dtype=np.float32)
    dison = np.zeros((N_CORES, 128, nblk), dtype=np.float32)
    disd = np.zeros((N_CORES, 128, s_pad), dtype=np.float32)
    for r in range(N_CORES):
        x_t[r, :, :s_real] = x[r * s_real : (r + 1) * s_real].T
        dv = np.zeros(s_pad, dtype=np.float32)
        dv[:s_real] = dis[r * s_real : (r + 1) * s_real]
        dison[r] = dv.reshape(nblk, 128).T
        disd[r] = dv[None, :]

    wdata = np.zeros((128, 3 * 128 + 3 + 128), dtype=np.float32)
    wdata[:, 0:128] = np.asarray(W0, dtype=np.float32)
    wdata[:, 128:256] = np.asarray(W1, dtype=np.float32)
    wdata[:, 256:384] = np.asarray(W2, dtype=np.float32)
    wdata[:, 384] = np.asarray(b0, dtype=np.float32)
    wdata[:, 385] = np.asarray(b1, dtype=np.float32)
    wdata[:, 386] = np.asarray(b2, dtype=np.float32)
    wdata[:, 387:515] = np.eye(128, dtype=np.float32)
    iotar = np.tile(
        np.arange(128, dtype=np.float32), WOH
    )[None, :].repeat(128, axis=0).astype(ml_dtypes.bfloat16)

    in_maps = [
        {
            "x_t": x_t[r], "meta": meta[r], "gidx": gidx[r],
            "wdata": wdata, "iotar": iotar, "dison": dison[r],
            "disd": disd[r],
        }
        for r in range(N_CORES)
    ]
    sched = {
        "nblk": nblk, "s_pad": s_pad, "s_real": s_real, "ngrp": ngrp,
        "caps32": caps32.tolist(),
    }
    return in_maps, sched


# ----------------------------------------------------------------------------
# Device kernel builder
# ----------------------------------------------------------------------------

def build_kernel(sched, n_cores=N_CORES):
    from contextlib import ExitStack

    nblk, s_pad, ngrp = sched["nblk"], sched["s_pad"], sched["ngrp"]
    caps32 = sched["caps32"]
    lay = _schedule(caps32, ngrp, nblk)
    win_rows = [c * 128 for c in WIN_BLKS]
    cbr = np.concatenate([[0], np.cumsum(win_rows)])
    cbb = np.concatenate([[0], np.cumsum(WIN_BLKS)])
    # window w's table rows complete after this group of the previous layer
    # (+2 for the split epilogue's deferred hm writes on non-tail groups)
    dep_group = [int(cbb[w + 1] + G - 1) // G - 1 for w in range(NW)]
    # mid-loop AG triggers with >=4 groups of slack so the gpsimd queue
    # never stalls on them; windows NW-2 / NW-1 issue after the loop / in
    # the next layer's pre-phase.
    ag_mid = {}
    for w in range(NW - 1):
        ag_mid[min(dep_group[w] + 6, ngrp - 1)] = w
    TAIL = ngrp - 9   # groups >= TAIL use the immediate epilogue

    nc = bacc.Bacc(
        "TRN2", target_bir_lowering=False, debug=False, num_devices=n_cores,
        num_swdge_queues=NW, dynamic_dma_scratch_size=SCRATCH,
    )
    x_t = nc.dram_tensor("x_t", [128, s_pad], F32, kind="ExternalInput")
    meta = nc.dram_tensor("meta", [128, lay["n_mm"]], BF16, kind="ExternalInput")
    gidx = nc.dram_tensor("gidx", [128, lay["gidx_cols"]], I16, kind="ExternalInput")
    wdata = nc.dram_tensor("wdata", [128, 3 * 128 + 3 + 128], F32, kind="ExternalInput")
    iotar = nc.dram_tensor("iotar", [128, WOH * 128], BF16, kind="ExternalInput")
    dison = nc.dram_tensor("dison", [128, nblk], F32, kind="ExternalInput")
    disd = nc.dram_tensor("disd", [128, s_pad], F32, kind="ExternalInput")
    h_out = nc.dram_tensor("h_out", [128, 3 * s_pad], F32, kind="ExternalOutput")

    rg = [list(range(n_cores))]
    ID = mybir.ActivationFunctionType

    with tile.TileContext(nc) as tc, ExitStack() as ctx:
        const = ctx.enter_context(tc.tile_pool(name="const", bufs=1))
        dram = ctx.enter_context(tc.tile_pool(name="dram", bufs=1, space="DRAM"))
        xw = ctx.enter_context(tc.tile_pool(name="xw", bufs=4))
        hmm = ctx.enter_context(tc.tile_pool(name="hmm", bufs=4))
        gath = ctx.enter_context(tc.tile_pool(name="gath", bufs=GATH_BUFS))
        ohp = ctx.enter_context(tc.tile_pool(name="ohp", bufs=10))
        hmo = ctx.enter_context(tc.tile_pool(name="hmo", bufs=12))
        outsb = ctx.enter_context(tc.tile_pool(name="outsb", bufs=3))
        ddp = ctx.enter_context(tc.tile_pool(name="ddp", bufs=3))
        rsb = ctx.enter_context(tc.tile_pool(name="rsb", bufs=2))
        agg_ps = ctx.enter_context(tc.tile_pool(name="agg_ps", bufs=2, space="PSUM"))
        mm_ps = ctx.enter_context(tc.tile_pool(name="mm_ps", bufs=2, space="PSUM"))
        mma_ps = ctx.enter_context(tc.tile_pool(name="mma_ps", bufs=2, space="PSUM"))

        ag_ins = [
            dram.tile([s_pad, 128], BF16, name=f"ag_in_l{i}") for i in range(3)
        ]
        ag_outs = [
            [
                dram.tile(
                    [n_cores * win_rows[w], 128], BF16,
                    addr_space="Shared", name=f"ag_out_l{i}_w{w}",
                )
                for w in range(NW)
            ]
            for i in range(3)
        ]

        w_sb = const.tile([128, 3 * 128 + 3 + 128], F32)
        nc.sync.dma_start(out=w_sb[:], in_=wdata[:])
        w_bf = const.tile([128, 3 * 128], BF16)
        nc.vector.tensor_copy(w_bf[:], w_sb[:, 0 : 3 * 128])
        id_bf = const.tile([128, 128], BF16)
        nc.vector.tensor_copy(id_bf[:], w_sb[:, 387:515])
        iota_sb = const.tile([128, WOH * 128], BF16)
        nc.sync.dma_start(out=iota_sb[:], in_=iotar[:])
        dison_sb = const.tile([128, nblk], F32)
        nc.sync.dma_start(out=dison_sb[:], in_=dison[:])
        # big index/meta constants load on the scalar HWDGE queue, deferred
        # into phase A (after the first window's hm writes) so they delay
        # neither the x_t slab loads (sync) nor the early hm writes (scalar)
        gidx_sb = const.tile([128, lay["gidx_cols"]], I16)
        meta_sb = const.tile([128, lay["n_mm"]], BF16)


        def bias(L):
            return w_sb[:, 384 + L : 385 + L]

        def issue_ag(L, w):
            r0, r1 = int(cbr[w]), int(cbr[w + 1])
            nc.gpsimd.collective_compute(
                "AllGather",
                mybir.AluOpType.bypass,
                replica_groups=rg,
                ins=[ag_ins[L][r0:r1, :].opt()],
                outs=[ag_outs[L][w][:].opt()],
            )

        def issue_gather(L, g, w, queue=None):
            nidx = lay["call_nidx"][g * NW + w]
            c0 = lay["call_cols"][g * NW + w]
            gt = gath.tile([128, nidx], BF16, name="gt", tag="gt")
            nc.gpsimd.dma_gather(
                gt[:].rearrange("p (c f) -> p c f", f=128),
                ag_outs[L][w][:],
                gidx_sb[:, c0 : c0 + nidx // 16],
                num_idxs=nidx,
                num_idxs_reg=nidx,
                elem_size=128,
                elem_step=128,
                single_packet=False,
                queue_num=w if queue is None else queue,
            )
            return gt

        # ---- Phase A: table0 = (x @ W0) * dis -> ag_in0, windowed AGs ----
        nslab = (nblk + 3) // 4
        ag_after_slab = {
            (int(cbb[w + 1]) + 3) // 4 - 1: w for w in range(NW - 1)
        }
        for sl in range(nslab):
            b0 = sl * 4
            bn = min(4, nblk - b0)
            ww = bn * 128
            xt = xw.tile([128, 512], F32, name="xt", tag="xt")
            nc.sync.dma_start(out=xt[:, :ww], in_=x_t[:, b0 * 128 : b0 * 128 + ww])
            ps = mma_ps.tile([128, 512], F32, name="psA", tag="psA")
            for j in range(bn):
                nc.tensor.matmul(
                    ps[:, j * 128 : (j + 1) * 128],
                    lhsT=xt[:, j * 128 : (j + 1) * 128],
                    rhs=w_sb[:, 0:128], start=True, stop=True,
                    skip_group_check=True,
                )
            hm = hmm.tile([128, 512], BF16, name="hmA", tag="hm")
            for j in range(bn):
                b = b0 + j
                nc.scalar.activation(
                    hm[:, j * 128 : (j + 1) * 128],
                    ps[:, j * 128 : (j + 1) * 128],
                    ID.Copy, scale=dison_sb[:, b : b + 1],
                )
                nc.scalar.dma_start(
                    out=ag_ins[0][b * 128 : (b + 1) * 128, :],
                    in_=hm[:, j * 128 : (j + 1) * 128],
                )
            if sl in ag_after_slab:
                issue_ag(0, ag_after_slab[sl])
            if sl == 7:
                nc.scalar.dma_start(out=gidx_sb[:], in_=gidx[:])
                nc.scalar.dma_start(out=meta_sb[:], in_=meta[:])

        # ---- 3 layers ----
        for L in range(3):
            # pre-phase: window-major issue of the first K_PRE groups for
            # desc-gen runway while the tail windows' AGs land
            pre = {g: [None] * NW for g in range(K_PRE)}
            for g in range(K_PRE):
                for w in (0, 1, 2):
                    pre[g][w] = issue_gather(L, g, w)
            issue_ag(L, NW - 1)   # this layer's last table window
            for g in range(K_PRE):
                pre[g][3] = issue_gather(L, g, 3, queue=(3 + g) % NW)

            state = {}

            def epi_a(gg):
                st = state[gg]
                ww, gb = st["nj"] * 128, gg * G * 128
                ob = outsb.tile([128, G * 128], F32, name="ob", tag="ob")
                nc.vector.tensor_tensor(
                    ob[:, :ww], st["ps"][:, :ww], st["dd"][:, :ww],
                    mybir.AluOpType.mult,
                )
                nc.scalar.activation(
                    ob[:, :ww], ob[:, :ww], ID.Identity, bias=bias(L)
                )
                nc.sync.dma_start(
                    out=h_out[:, L * s_pad + gb : L * s_pad + gb + ww],
                    in_=ob[:, :ww],
                )
                st["ob"] = ob

            def epi_b(gg):
                st = state.pop(gg)
                if L >= 2:
                    return
                ww = st["nj"] * 128
                ob = st["ob"]
                r = rsb.tile([128, G * 128], BF16, name="r", tag="r")
                nc.scalar.activation(r[:, :ww], ob[:, :ww], ID.Relu)
                ps2 = mm_ps.tile([128, G * 128], F32, name="ps2", tag="ps2")
                for j in range(st["nj"]):
                    nc.tensor.matmul(
                        ps2[:, j * 128 : (j + 1) * 128],
                        lhsT=r[:, j * 128 : (j + 1) * 128],
                        rhs=w_bf[:, (L + 1) * 128 : (L + 2) * 128],
                        start=True,
                        stop=True,
                        skip_group_check=True,
                    )
                hm = hmm.tile([128, G * 128], BF16, name="hm", tag="hm")
                for j in range(st["nj"]):
                    b = st["blocks"][j]
                    nc.scalar.activation(
                        hm[:, j * 128 : (j + 1) * 128],
                        ps2[:, j * 128 : (j + 1) * 128], ID.Copy,
                        scale=dison_sb[:, b : b + 1],
                    )
                    nc.scalar.dma_start(
                        out=ag_ins[L + 1][b * 128 : (b + 1) * 128, :],
                        in_=hm[:, j * 128 : (j + 1) * 128],
                    )

            for g in range(ngrp):
                blocks = list(range(g * G, min((g + 1) * G, nblk)))
                nj = len(blocks)
                gts = pre[g] if g < K_PRE else [
                    issue_gather(L, g, w) for w in range(NW)
                ]
                if L < 2 and g in ag_mid:
                    issue_ag(L + 1, ag_mid[g])

                mms = lay["mm_of_group"][g]
                m0 = int(lay["mm_base"][g])
                n_mm_g = len(mms)

                ohs = {}
                for w0 in range(0, n_mm_g, WOH):
                    wn = min(WOH, n_mm_g - w0)
                    oh = ohp.tile([128, wn * 128], BF16, name="oh", tag="oh")
                    nc.vector.tensor_tensor(
                        oh[:].rearrange("p (c f) -> p c f", f=128),
                        iota_sb[:, : wn * 128].rearrange("p (c f) -> p c f", f=128),
                        meta_sb[:, m0 + w0 : m0 + w0 + wn].to_broadcast(
                            [128, wn, 128]
                        ),
                        mybir.AluOpType.is_equal,
                    )
                    ohs[w0] = oh

                # own-table rows for the self-loop identity matmuls
                hms = []
                for j in range(nj):
                    b = blocks[j]
                    hmj = hmo.tile([128, 128], BF16, name="hmo", tag="hmo")
                    nc.sync.dma_start(
                        out=hmj[:], in_=ag_ins[L][b * 128 : (b + 1) * 128, :]
                    )
                    hms.append(hmj)

                # deferred epilogue A for the previous group
                if g - 1 >= 0 and g - 1 < TAIL:
                    epi_a(g - 1)

                dd = ddp.tile([128, G * 128], F32, name="dd", tag="dd")
                nc.scalar.dma_start(
                    out=dd[:, : nj * 128],
                    in_=disd[:, g * G * 128 : g * G * 128 + nj * 128],
                )
                ps = agg_ps.tile([128, G * 128], F32, name="aggps", tag="aggps")
                # strictly sequential PSUM brackets: per block j, open with
                # the self-loop identity matmul, accumulate j's edge mms,
                # close on the last one.
                ptr = 0
                for j in range(nj):
                    nc.tensor.matmul(
                        ps[:, j * 128 : (j + 1) * 128],
                        lhsT=hms[j][:],
                        rhs=id_bf[:],
                        start=True,
                        stop=False,
                        skip_group_check=True,
                    )
                    while ptr < len(mms) and mms[ptr][0] == j:
                        k = ptr
                        _, ww_, c = mms[k]
                        w0 = (k // WOH) * WOH
                        off = k - w0
                        last = (k + 1 == len(mms)) or (mms[k + 1][0] != j)
                        nc.tensor.matmul(
                            ps[:, j * 128 : (j + 1) * 128],
                            lhsT=gts[ww_][:, c * 128 : (c + 1) * 128],
                            rhs=ohs[w0][:, off * 128 : (off + 1) * 128],
                            start=False,
                            stop=last,
                            skip_group_check=True,
                        )
                        ptr += 1
                state[g] = {"ps": ps, "dd": dd, "nj": nj, "blocks": blocks}

                # deferred epilogue B two groups back
                if g - 2 >= 0 and g - 2 < TAIL:
                    epi_b(g - 2)
                # tail groups: immediate epilogue so hm rows land promptly
                if g >= TAIL:
                    epi_a(g)
                    epi_b(g)
            # flush the last deferred groups (TAIL-2, TAIL-1)
            for gg in (TAIL - 2, TAIL - 1):
                if gg in state:
                    if "ob" not in state[gg]:
                        epi_a(gg)
                    epi_b(gg)

    nc.compile()
    return nc


_BUILD_CACHE = {}


def _get_kernel(sched):
    key = (
        sched["nblk"], sched["s_pad"],
        tuple(tuple(c) for c in sched["caps32"]),
    )
    if key not in _BUILD_CACHE:
        _BUILD_CACHE[key] = build_kernel(sched)
    return _BUILD_CACHE[key]


# ----------------------------------------------------------------------------
# Entry point
# ----------------------------------------------------------------------------

def _run(x, edge_index, W0, b0, W1, b1, W2, b2, trace=False):
    n = int(np.asarray(x).shape[0])
    s_real = n // N_CORES
    in_maps, sched = _prep_inputs(
        x, edge_index, W0, b0, W1, b1, W2, b2, s_real
    )
    s_pad = sched["s_pad"]
    nc = _get_kernel(sched)
    res = bass_utils.run_bass_kernel_spmd(
        nc, in_maps, core_ids=list(range(N_CORES)), trace=trace
    )
    outs = []
    for L in range(3):
        h = np.concatenate(
            [
                res.results[r]["h_out"][:, L * s_pad : L * s_pad + s_real]
                for r in range(N_CORES)
            ],
            axis=1,
        ).T
        outs.append(h)
    full = np.stack(outs, axis=1).astype(np.float32)
    return full, res


def kernel(**inputs):
    trace = os.environ.get("TRN_KERNEL_TRACE", "") == "1"
    out, res = _run(
        np.asarray(inputs["x"]),
        np.asarray(inputs["edge_index"]),
        np.asarray(inputs["W0"]),
        np.asarray(inputs["b0"]),
        np.asarray(inputs["W1"]),
        np.asarray(inputs["b1"]),
        np.asarray(inputs["W2"]),
        np.asarray(inputs["b2"]),
        trace=trace,
    )
    if trace and res.exec_time_ns is not None:
        print(f"HW exec time: {res.exec_time_ns} ns")
        if res.instructions_and_trace:
            print(f"trace: {res.instructions_and_trace[1]}")
    return out


# revision 24
# speedup vs baseline: 1.1244x; 1.1244x over previous
"""3-layer GCN (gcn_norm message passing) on 8 Trainium2 NeuronCores.

Architecture (v8):
  - Nodes row-sharded across 8 cores (12500 real + 44 pad rows each); per
    layer each core computes h_mm = relu(h_prev) @ W for its shard, scaled by
    dis[src] (norm factorization: norm = dis[dest]*dis[src]), AllGathers the
    bf16 table, then aggregates messages for the destinations it owns.
  - The per-layer AllGather is split into 4 source windows of [28,28,28,14]
    blocks (int16 gather-index range bounds a window at 8*31 blocks); window
    w's collective fires mid-previous-layer once its table rows are written,
    with enough group slack that the trigger never stalls the gpsimd queue.
    Gathers for window w wait only on window w's collective; the tail
    window is small to soften layer boundaries.
  - The SWDGE gather desc-gen pipe sustains ~2.8ns/descriptor regardless of
    call size; calls are per (PSUM group, window) (matches compute
    granularity), with a window-major pre-issue of the first 3 groups at
    each layer start for pipe runway while tail-window collectives land.
  - Self-loops are NOT gathered: each block's own table rows are added into
    the aggregation PSUM with an identity matmul opening that block's PSUM
    bracket (hm rows are already scaled by dis[src]; the epilogue's
    *dis[dest] completes the dis^2 self norm).
  - Segment-sum on the TensorEngine: each 128-message chunk contributes one
    N=128 matmul per (statically known) destination block it overlaps, into
    a [128, 512] group PSUM tile with strictly sequential per-block
    accumulation brackets; one-hots are 0/1 bf16 built 8 chunks at a time
    with a single wide DVE tensor_tensor against a bf16 iota.
  - Split epilogue pipelining: group g's scale/bias/h_out run one group
    later and relu/next-layer-matmul/hm two groups later, so the in-order
    DVE/PE/ACT queues never convoy (one-hot builds for g+1 are queued ahead
    of g's PSUM-dependent ops). Tail groups use an immediate epilogue so the
    next layer's collectives see their table rows promptly.
  - gidx and meta live in SBUF (loaded once, sliced per call).

All data-dependent structure is baked at trace time; the NEFF is compiled
per call and cached in-process.
"""

import os
import sys

sys.path.insert(0, "/opt/trn_rl_repo")

import numpy as np
import ml_dtypes

from concourse import bacc, bass, mybir
from concourse import tile
from concourse import bass_utils

F32 = mybir.dt.float32
BF16 = mybir.dt.bfloat16
I16 = mybir.dt.int16

N_CORES = 8
G = 4        # dest blocks per PSUM group
WOH = 8      # one-hot chunks per wide DVE op
RUN = 32     # run alignment granularity (slots)
PAD_SEG = 10000.0
WIN_BLKS = [28, 28, 28, 14]   # source blocks per window (sum = nblk)
NW = len(WIN_BLKS)
K_PRE = 4                      # groups pre-issued at each layer start

SCRATCH = int(os.environ.get("TRN_SCRATCH", "32768"))
GATH_BUFS = int(os.environ.get("TRN_GATH_BUFS", "16"))


def _schedule(caps32, ngrp, nblk):
    """Static layout shared by prep and builder.

    caps32: [nblk][NW] per-(block, window) run capacity in RUN-slot units.
    Returns per-call offsets and the chunk->block matmul schedule.
    """
    call_cols = np.zeros(ngrp * NW + 1, dtype=np.int64)   # gidx col base
    chunk_base = np.zeros(ngrp * NW + 1, dtype=np.int64)  # chunk id base
    call_nidx = []
    run_slot = {}   # (b, w) -> slot offset of run inside its call
    mm_of_group = []
    for g in range(ngrp):
        blocks = list(range(g * G, min((g + 1) * G, nblk)))
        mms = []
        for w in range(NW):
            off = 0
            spans = []
            for b in blocks:
                run_slot[(b, w)] = off
                spans.append((b, off, off + caps32[b][w] * RUN))
                off += caps32[b][w] * RUN
            nidx = ((off + 127) // 128) * 128
            call_nidx.append(nidx)
            call_cols[g * NW + w + 1] = call_cols[g * NW + w] + nidx // 16
            chunk_base[g * NW + w + 1] = chunk_base[g * NW + w] + nidx // 128
            for c in range(nidx // 128):
                lo, hi = c * 128, (c + 1) * 128
                for b, s0, s1 in spans:
                    if s0 < hi and s1 > lo:
                        mms.append((b - g * G, w, c))
        # j-major order for PSUM accumulation bracketing
        mms.sort(key=lambda m: (m[0], m[1], m[2]))
        mm_of_group.append(mms)
    mm_base = np.zeros(ngrp + 1, dtype=np.int64)
    np.cumsum([len(m) for m in mm_of_group], out=mm_base[1:])
    return {
        "call_cols": call_cols, "chunk_base": chunk_base,
        "call_nidx": call_nidx, "run_slot": run_slot,
        "mm_of_group": mm_of_group, "mm_base": mm_base,
        "n_mm": int(mm_base[-1]),
        "gidx_cols": int(call_cols[-1]),
        "total_chunks": int(chunk_base[-1]),
    }


# ----------------------------------------------------------------------------
# Host-side preparation
# ----------------------------------------------------------------------------

def _prep_inputs(x, edge_index, W0, b0, W1, b1, W2, b2, s_real):
    n = x.shape[0]
    assert n % N_CORES == 0 and s_real == n // N_CORES
    nblk = (s_real + 127) // 128
    s_pad = nblk * 128
    ngrp = (nblk + G - 1) // G
    assert sum(WIN_BLKS) == nblk
    win_rows = [c * 128 for c in WIN_BLKS]
    cbr = np.concatenate([[0], np.cumsum(win_rows)])
    cbb = np.concatenate([[0], np.cumsum(WIN_BLKS)])
    assert all(N_CORES * r <= 32767 for r in win_rows)
    assert all(b % G == 0 for b in cbb[:-1])

    d = np.asarray(edge_index[0], dtype=np.int64)
    s = np.asarray(edge_index[1], dtype=np.int64)

    deg = np.bincount(s, minlength=n).astype(np.float64) + 1.0
    dis = (1.0 / np.sqrt(deg)).astype(np.float32)

    core = d // s_real
    dloc = d - core * s_real
    blk = dloc >> 7
    grp = blk // G
    jj = blk - grp * G
    score = s // s_real
    sloc = s - score * s_real
    sblk = sloc >> 7
    blk2win = np.zeros(nblk, dtype=np.int64)
    for w in range(NW):
        blk2win[cbb[w]:cbb[w + 1]] = w
    q = blk2win[sblk]
    widx = (
        score * np.asarray(win_rows)[q] + (sloc - cbr[q])
    ).astype(np.int64)

    # per-(core, block, window) counts -> shared run capacities (RUN units)
    key = (core * nblk + blk) * NW + q
    counts = np.bincount(key, minlength=N_CORES * nblk * NW).reshape(
        N_CORES, nblk, NW
    )
    caps32 = np.maximum(
        (counts.max(axis=0) + RUN - 1) // RUN, 1
    )  # [nblk, NW]

    lay = _schedule(caps32.tolist(), ngrp, nblk)

    # rank within (core, b, w)
    order = np.argsort(key, kind="stable")
    inv = np.empty_like(order)
    inv[order] = np.arange(order.size)
    starts = np.zeros(N_CORES * nblk * NW + 1, dtype=np.int64)
    np.cumsum(counts.reshape(-1), out=starts[1:])
    rank = inv - starts[key]

    run_slot_arr = np.zeros((nblk, NW), dtype=np.int64)
    for (b, ww), v in lay["run_slot"].items():
        run_slot_arr[b, ww] = v

    slot = run_slot_arr[blk, q] + rank               # slot within call
    call_id = grp * NW + q
    gcol = lay["call_cols"][call_id] + (slot >> 4)
    grow = slot & 15
    gchunk = lay["chunk_base"][call_id] + (slot >> 7)
    part = slot & 127

    gidx16 = np.zeros((N_CORES, 16, lay["gidx_cols"]), dtype=np.int16)
    gidx16[core, grow, gcol] = widx.astype(np.int16)
    gidx = np.broadcast_to(
        gidx16[:, None, :, :], (N_CORES, 8, 16, lay["gidx_cols"])
    ).reshape(N_CORES, 128, lay["gidx_cols"]).copy()

    # mm col lookup: (gchunk, j) -> column
    mm_col = np.full((lay["total_chunks"], G), -1, dtype=np.int64)
    cb = lay["chunk_base"]
    for g in range(ngrp):
        m0 = lay["mm_base"][g]
        for k, (j, ww, c) in enumerate(lay["mm_of_group"][g]):
            mm_col[cb[g * NW + ww] + c, j] = m0 + k

    meta = np.full((N_CORES, 128, lay["n_mm"]), PAD_SEG, dtype=np.float32)
    col = mm_col[gchunk, jj]
    assert (col >= 0).all()
    meta[core, part, col] = (dloc - blk * 128).astype(np.float32)
    meta = meta.astype(ml_dtypes.bfloat16)

    # dense inputs
    x = np.asarray(x, dtype=np.float32)
    x_t = np.zeros((N_CORES, 128, s_pad), dtype=np.float32)
    dison = np.zeros((N_CORES, 128, nblk), dtype=np.float32)
    disd = np.zeros((N_CORES, 128, s_pad), dtype=np.float32)
    for r in range(N_CORES):
        x_t[r, :, :s_real] = x[r * s_real : (r + 1) * s_real].T
        dv = np.zeros(s_pad, dtype=np.float32)
        dv[:s_real] = dis[r * s_real : (r + 1) * s_real]
        dison[r] = dv.reshape(nblk, 128).T
        disd[r] = dv[None, :]

    wdata = np.zeros((128, 3 * 128 + 3 + 128), dtype=np.float32)
    wdata[:, 0:128] = np.asarray(W0, dtype=np.float32)
    wdata[:, 128:256] = np.asarray(W1, dtype=np.float32)
    wdata[:, 256:384] = np.asarray(W2, dtype=np.float32)
    wdata[:, 384] = np.asarray(b0, dtype=np.float32)
    wdata[:, 385] = np.asarray(b1, dtype=np.float32)
    wdata[:, 386] = np.asarray(b2, dtype=np.float32)
    wdata[:, 387:515] = np.eye(128, dtype=np.float32)
    iotar = np.tile(
        np.arange(128, dtype=np.float32), WOH
    )[None, :].repeat(128, axis=0).astype(ml_dtypes.bfloat16)

    in_maps = [
        {
            "x_t": x_t[r], "meta": meta[r], "gidx": gidx[r],
            "wdata": wdata, "iotar": iotar, "dison": dison[r],
            "disd": disd[r],
        }
        for r in range(N_CORES)
    ]
    sched = {
        "nblk": nblk, "s_pad": s_pad, "s_real": s_real, "ngrp": ngrp,
        "caps32": caps32.tolist(),
    }
    return in_maps, sched


# ----------------------------------------------------------------------------
# Device kernel builder
# ----------------------------------------------------------------------------

def build_kernel(sched, n_cores=N_CORES):
    from contextlib import ExitStack

    nblk, s_pad, ngrp = sched["nblk"], sched["s_pad"], sched["ngrp"]
    caps32 = sched["caps32"]
    lay = _schedule(caps32, ngrp, nblk)
    win_rows = [c * 128 for c in WIN_BLKS]
    cbr = np.concatenate([[0], np.cumsum(win_rows)])
    cbb = np.concatenate([[0], np.cumsum(WIN_BLKS)])
    # window w's table rows complete after this group of the previous layer
    # (+2 for the split epilogue's deferred hm writes on non-tail groups)
    dep_group = [int(cbb[w + 1] + G - 1) // G - 1 for w in range(NW)]
    # mid-loop AG triggers with >=4 groups of slack so the gpsimd queue
    # never stalls on them; windows NW-2 / NW-1 issue after the loop / in
    # the next layer's pre-phase.
    ag_mid = {}
    for w in range(NW - 1):
        ag_mid[min(dep_group[w] + 6, ngrp - 1)] = w
    TAIL = ngrp - 5   # groups >= TAIL use the immediate epilogue

    nc = bacc.Bacc(
        "TRN2", target_bir_lowering=False, debug=False, num_devices=n_cores,
        num_swdge_queues=NW, dynamic_dma_scratch_size=SCRATCH,
    )
    x_t = nc.dram_tensor("x_t", [128, s_pad], F32, kind="ExternalInput")
    meta = nc.dram_tensor("meta", [128, lay["n_mm"]], BF16, kind="ExternalInput")
    gidx = nc.dram_tensor("gidx", [128, lay["gidx_cols"]], I16, kind="ExternalInput")
    wdata = nc.dram_tensor("wdata", [128, 3 * 128 + 3 + 128], F32, kind="ExternalInput")
    iotar = nc.dram_tensor("iotar", [128, WOH * 128], BF16, kind="ExternalInput")
    dison = nc.dram_tensor("dison", [128, nblk], F32, kind="ExternalInput")
    disd = nc.dram_tensor("disd", [128, s_pad], F32, kind="ExternalInput")
    h_out = nc.dram_tensor("h_out", [128, 3 * s_pad], F32, kind="ExternalOutput")

    rg = [list(range(n_cores))]
    ID = mybir.ActivationFunctionType

    with tile.TileContext(nc) as tc, ExitStack() as ctx:
        const = ctx.enter_context(tc.tile_pool(name="const", bufs=1))
        dram = ctx.enter_context(tc.tile_pool(name="dram", bufs=1, space="DRAM"))
        xw = ctx.enter_context(tc.tile_pool(name="xw", bufs=4))
        hmm = ctx.enter_context(tc.tile_pool(name="hmm", bufs=4))
        gath = ctx.enter_context(tc.tile_pool(name="gath", bufs=GATH_BUFS))
        ohp = ctx.enter_context(tc.tile_pool(name="ohp", bufs=12))
        hmo = ctx.enter_context(tc.tile_pool(name="hmo", bufs=12))
        outsb = ctx.enter_context(tc.tile_pool(name="outsb", bufs=3))
        ddp = ctx.enter_context(tc.tile_pool(name="ddp", bufs=3))
        rsb = ctx.enter_context(tc.tile_pool(name="rsb", bufs=2))
        agg_ps = ctx.enter_context(tc.tile_pool(name="agg_ps", bufs=2, space="PSUM"))
        mm_ps = ctx.enter_context(tc.tile_pool(name="mm_ps", bufs=2, space="PSUM"))
        mma_ps = ctx.enter_context(tc.tile_pool(name="mma_ps", bufs=2, space="PSUM"))

        ag_ins = [
            dram.tile([s_pad, 128], BF16, name=f"ag_in_l{i}") for i in range(3)
        ]
        ag_outs = [
            [
                dram.tile(
                    [n_cores * win_rows[w], 128], BF16,
                    addr_space="Shared", name=f"ag_out_l{i}_w{w}",
                )
                for w in range(NW)
            ]
            for i in range(3)
        ]

        w_sb = const.tile([128, 3 * 128 + 3 + 128], F32)
        nc.sync.dma_start(out=w_sb[:], in_=wdata[:])
        w_bf = const.tile([128, 3 * 128], BF16)
        nc.vector.tensor_copy(w_bf[:], w_sb[:, 0 : 3 * 128])
        id_bf = const.tile([128, 128], BF16)
        nc.vector.tensor_copy(id_bf[:], w_sb[:, 387:515])
        iota_sb = const.tile([128, WOH * 128], BF16)
        nc.sync.dma_start(out=iota_sb[:], in_=iotar[:])
        dison_sb = const.tile([128, nblk], F32)
        nc.sync.dma_start(out=dison_sb[:], in_=dison[:])
        # big index/meta constants load on the scalar HWDGE queue, deferred
        # into phase A (after the first window's hm writes) so they delay
        # neither the x_t slab loads (sync) nor the early hm writes (scalar)
        gidx_sb = const.tile([128, lay["gidx_cols"]], I16)
        meta_sb = const.tile([128, lay["n_mm"]], BF16)


        def bias(L):
            return w_sb[:, 384 + L : 385 + L]

        def issue_ag(L, w):
            r0, r1 = int(cbr[w]), int(cbr[w + 1])
            nc.gpsimd.collective_compute(
                "AllGather",
                mybir.AluOpType.bypass,
                replica_groups=rg,
                ins=[ag_ins[L][r0:r1, :].opt()],
                outs=[ag_outs[L][w][:].opt()],
            )

        def issue_gather(L, g, w, queue=None):
            nidx = lay["call_nidx"][g * NW + w]
            c0 = lay["call_cols"][g * NW + w]
            gt = gath.tile([128, nidx], BF16, name="gt", tag="gt")
            nc.gpsimd.dma_gather(
                gt[:].rearrange("p (c f) -> p c f", f=128),
                ag_outs[L][w][:],
                gidx_sb[:, c0 : c0 + nidx // 16],
                num_idxs=nidx,
                num_idxs_reg=nidx,
                elem_size=128,
                elem_step=128,
                single_packet=False,
                queue_num=w if queue is None else queue,
            )
            return gt

        # ---- Phase A: table0 = (x @ W0) * dis -> ag_in0, windowed AGs ----
        nslab = (nblk + 3) // 4
        ag_after_slab = {
            (int(cbb[w + 1]) + 3) // 4 - 1: w for w in range(NW - 1)
        }
        for sl in range(nslab):
            b0 = sl * 4
            bn = min(4, nblk - b0)
            ww = bn * 128
            xt = xw.tile([128, 512], F32, name="xt", tag="xt")
            nc.sync.dma_start(out=xt[:, :ww], in_=x_t[:, b0 * 128 : b0 * 128 + ww])
            ps = mma_ps.tile([128, 512], F32, name="psA", tag="psA")
            for j in range(bn):
                nc.tensor.matmul(
                    ps[:, j * 128 : (j + 1) * 128],
                    lhsT=xt[:, j * 128 : (j + 1) * 128],
                    rhs=w_sb[:, 0:128], start=True, stop=True,
                    skip_group_check=True,
                )
            hm = hmm.tile([128, 512], BF16, name="hmA", tag="hm")
            for j in range(bn):
                b = b0 + j
                nc.scalar.activation(
                    hm[:, j * 128 : (j + 1) * 128],
                    ps[:, j * 128 : (j + 1) * 128],
                    ID.Copy, scale=dison_sb[:, b : b + 1],
                )
                nc.scalar.dma_start(
                    out=ag_ins[0][b * 128 : (b + 1) * 128, :],
                    in_=hm[:, j * 128 : (j + 1) * 128],
                )
            if sl in ag_after_slab:
                issue_ag(0, ag_after_slab[sl])
            if sl == 7:
                nc.scalar.dma_start(out=gidx_sb[:], in_=gidx[:])
                nc.scalar.dma_start(out=meta_sb[:], in_=meta[:])

        # ---- 3 layers ----
        for L in range(3):
            # pre-phase: window-major issue of the first K_PRE groups for
            # desc-gen runway while the tail windows' AGs land
            pre = {g: [None] * NW for g in range(K_PRE)}
            for g in range(K_PRE):
                for w in (0, 1, 2):
                    pre[g][w] = issue_gather(L, g, w)
            issue_ag(L, NW - 1)   # this layer's last table window
            for g in range(K_PRE):
                pre[g][3] = issue_gather(L, g, 3, queue=(3 + g) % NW)

            state = {}

            def epi_a(gg):
                st = state[gg]
                ww, gb = st["nj"] * 128, gg * G * 128
                ob = outsb.tile([128, G * 128], F32, name="ob", tag="ob")
                nc.vector.tensor_tensor(
                    ob[:, :ww], st["ps"][:, :ww], st["dd"][:, :ww],
                    mybir.AluOpType.mult,
                )
                nc.scalar.activation(
                    ob[:, :ww], ob[:, :ww], ID.Identity, bias=bias(L)
                )
                nc.sync.dma_start(
                    out=h_out[:, L * s_pad + gb : L * s_pad + gb + ww],
                    in_=ob[:, :ww],
                )
                st["ob"] = ob

            def epi_b(gg):
                st = state.pop(gg)
                if L >= 2:
                    return
                ww = st["nj"] * 128
                ob = st["ob"]
                r = rsb.tile([128, G * 128], BF16, name="r", tag="r")
                nc.scalar.activation(r[:, :ww], ob[:, :ww], ID.Relu)
                ps2 = mm_ps.tile([128, G * 128], F32, name="ps2", tag="ps2")
                for j in range(st["nj"]):
                    nc.tensor.matmul(
                        ps2[:, j * 128 : (j + 1) * 128],
                        lhsT=r[:, j * 128 : (j + 1) * 128],
                        rhs=w_bf[:, (L + 1) * 128 : (L + 2) * 128],
                        start=True,
                        stop=True,
                        skip_group_check=True,
                    )
                hm = hmm.tile([128, G * 128], BF16, name="hm", tag="hm")
                for j in range(st["nj"]):
                    b = st["blocks"][j]
                    nc.scalar.activation(
                        hm[:, j * 128 : (j + 1) * 128],
                        ps2[:, j * 128 : (j + 1) * 128], ID.Copy,
                        scale=dison_sb[:, b : b + 1],
                    )
                    nc.scalar.dma_start(
                        out=ag_ins[L + 1][b * 128 : (b + 1) * 128, :],
                        in_=hm[:, j * 128 : (j + 1) * 128],
                    )

            for g in range(ngrp):
                blocks = list(range(g * G, min((g + 1) * G, nblk)))
                nj = len(blocks)
                gts = pre[g] if g < K_PRE else [
                    issue_gather(L, g, w) for w in range(NW)
                ]
                if L < 2 and g in ag_mid:
                    issue_ag(L + 1, ag_mid[g])

                mms = lay["mm_of_group"][g]
                m0 = int(lay["mm_base"][g])
                n_mm_g = len(mms)

                ohs = {}
                for w0 in range(0, n_mm_g, WOH):
                    wn = min(WOH, n_mm_g - w0)
                    oh = ohp.tile([128, wn * 128], BF16, name="oh", tag="oh")
                    nc.vector.tensor_tensor(
                        oh[:].rearrange("p (c f) -> p c f", f=128),
                        iota_sb[:, : wn * 128].rearrange("p (c f) -> p c f", f=128),
                        meta_sb[:, m0 + w0 : m0 + w0 + wn].to_broadcast(
                            [128, wn, 128]
                        ),
                        mybir.AluOpType.is_equal,
                    )
                    ohs[w0] = oh

                # own-table rows for the self-loop identity matmuls
                hms = []
                for j in range(nj):
                    b = blocks[j]
                    hmj = hmo.tile([128, 128], BF16, name="hmo", tag="hmo")
                    nc.sync.dma_start(
                        out=hmj[:], in_=ag_ins[L][b * 128 : (b + 1) * 128, :]
                    )
                    hms.append(hmj)

                # deferred epilogue A for the previous group
                if g - 1 >= 0 and g - 1 < TAIL:
                    epi_a(g - 1)

                dd = ddp.tile([128, G * 128], F32, name="dd", tag="dd")
                nc.scalar.dma_start(
                    out=dd[:, : nj * 128],
                    in_=disd[:, g * G * 128 : g * G * 128 + nj * 128],
                )
                ps = agg_ps.tile([128, G * 128], F32, name="aggps", tag="aggps")
                # strictly sequential PSUM brackets: per block j, open with
                # the self-loop identity matmul, accumulate j's edge mms,
                # close on the last one.
                ptr = 0
                for j in range(nj):
                    nc.tensor.matmul(
                        ps[:, j * 128 : (j + 1) * 128],
                        lhsT=hms[j][:],
                        rhs=id_bf[:],
                        start=True,
                        stop=False,
                        skip_group_check=True,
                    )
                    while ptr < len(mms) and mms[ptr][0] == j:
                        k = ptr
                        _, ww_, c = mms[k]
                        w0 = (k // WOH) * WOH
                        off = k - w0
                        last = (k + 1 == len(mms)) or (mms[k + 1][0] != j)
                        nc.tensor.matmul(
                            ps[:, j * 128 : (j + 1) * 128],
                            lhsT=gts[ww_][:, c * 128 : (c + 1) * 128],
                            rhs=ohs[w0][:, off * 128 : (off + 1) * 128],
                            start=False,
                            stop=last,
                            skip_group_check=True,
                        )
                        ptr += 1
                state[g] = {"ps": ps, "dd": dd, "nj": nj, "blocks": blocks}

                # deferred epilogue B two groups back
                if g - 2 >= 0 and g - 2 < TAIL:
                    epi_b(g - 2)
                # tail groups: immediate epilogue so hm rows land promptly
                if g >= TAIL:
                    epi_a(g)
                    epi_b(g)
            # flush the last deferred groups (TAIL-2, TAIL-1)
            for gg in (TAIL - 2, TAIL - 1):
                if gg in state:
                    if "ob" not in state[gg]:
                        epi_a(gg)
                    epi_b(gg)

    nc.compile()
    return nc


_BUILD_CACHE = {}


def _get_kernel(sched):
    key = (
        sched["nblk"], sched["s_pad"],
        tuple(tuple(c) for c in sched["caps32"]),
    )
    if key not in _BUILD_CACHE:
        _BUILD_CACHE[key] = build_kernel(sched)
    return _BUILD_CACHE[key]


# ----------------------------------------------------------------------------
# Entry point
# ----------------------------------------------------------------------------

def _run(x, edge_index, W0, b0, W1, b1, W2, b2, trace=False):
    n = int(np.asarray(x).shape[0])
    s_real = n // N_CORES
    in_maps, sched = _prep_inputs(
        x, edge_index, W0, b0, W1, b1, W2, b2, s_real
    )
    s_pad = sched["s_pad"]
    nc = _get_kernel(sched)
    res = bass_utils.run_bass_kernel_spmd(
        nc, in_maps, core_ids=list(range(N_CORES)), trace=trace
    )
    outs = []
    for L in range(3):
        h = np.concatenate(
            [
                res.results[r]["h_out"][:, L * s_pad : L * s_pad + s_real]
                for r in range(N_CORES)
            ],
            axis=1,
        ).T
        outs.append(h)
    full = np.stack(outs, axis=1).astype(np.float32)
    return full, res


def kernel(**inputs):
    trace = os.environ.get("TRN_KERNEL_TRACE", "") == "1"
    out, res = _run(
        np.asarray(inputs["x"]),
        np.asarray(inputs["edge_index"]),
        np.asarray(inputs["W0"]),
        np.asarray(inputs["b0"]),
        np.asarray(inputs["W1"]),
        np.asarray(inputs["b1"]),
        np.asarray(inputs["W2"]),
        np.asarray(inputs["b2"]),
        trace=trace,
    )
    if trace and res.exec_time_ns is not None:
        print(f"HW exec time: {res.exec_time_ns} ns")
        if res.instructions_and_trace:
            print(f"trace: {res.instructions_and_trace[1]}")
    return out
